# revision 7
# baseline (speedup 1.0000x reference)
"""Trainium2 Bass kernel for sparse (causal, tanh-clamped) attention.

Problem: B=2, L=2048, D=1024, H=16 heads x 64 dim; S = QK^T/8;
S = 30*tanh(S); causal + attention_mask; softmax; out = attn @ V.

Sharding: 2 heads per core across 8 cores (tensor-parallel on heads).
Each core computes its 128 output features for the full batch.

Key design points:
 - All matmuls run in float32r (TF32-like, 1 cyc/row on PE for moving
   dim >= 256; HW rounds fp32 inputs internally).
 - Everything is computed in the transposed layout S^T[k, q] so that no
   P-matrix transpose is needed: S^T = K_aug^T @ Q_aug with the
   contraction (d) on partitions, softmax numerator P^T feeds the AV
   matmul directly as the moving operand.
 - attention_mask is folded into the score matmul via an augmented 65th
   contraction row: K row 64 = (mask-1)*1e6, Q row 64 = 1.  tanh then
   saturates masked scores to -1 -> P = e^-60 ~ 0.
 - Bounded logits (30*tanh in [-30, 30]) mean softmax needs no running
   max: P = exp(30*tanh(s) - 30) in (0, 1]; denominator comes for free
   as a ones-column appended to V in the AV matmul.
 - Causal masking: per k-tile the q range starts at the diagonal, and
   only the 128x128 diagonal block needs a tril multiply on P.
"""

import sys

if "/opt/trn_rl_repo" not in sys.path:
    sys.path.insert(0, "/opt/trn_rl_repo")

import numpy as np

B = 2
L = 2048
D = 1024
H = 16
DH = 64
N_CORES = 8
T = B * L            # 4096 tokens
E = 128              # per-core output features (2 heads)
NEG_BIG = 1.0e6      # mask additive; tanh saturates anything big
TAU = 30.0

_CACHE = {}


def _build_program():
    import concourse.bacc as bacc
    import concourse.tile as tile
    from concourse import mybir

    F32 = mybir.dt.float32
    F32R = mybir.dt.float32r
    AF = mybir.ActivationFunctionType

    nc = bacc.Bacc("TRN2", target_bir_lowering=False, debug=False,
                   num_devices=N_CORES)

    xT_d = nc.dram_tensor("xT", [D, T], F32R, kind="ExternalInput")
    wq_d = nc.dram_tensor("wq", [D, E], F32R, kind="ExternalInput")
    wk_d = nc.dram_tensor("wk", [D, E], F32R, kind="ExternalInput")
    wv_d = nc.dram_tensor("wv", [D, E], F32R, kind="ExternalInput")
    kaug_d = nc.dram_tensor("kaug", [1, T], F32R, kind="ExternalInput")
    ones_d = nc.dram_tensor("onesrow", [1, T], F32R, kind="ExternalInput")
    onescol_d = nc.dram_tensor("onescol", [128, 1], F32R, kind="ExternalInput")
    tril_d = nc.dram_tensor("tril", [128, 128], F32, kind="ExternalInput")
    ident_d = nc.dram_tensor("ident", [128, 128], F32R, kind="ExternalInput")
    out_d = nc.dram_tensor("out", [B, L, E], F32, kind="ExternalOutput")

    ND = D // 128        # 8 contraction chunks for projections
    NT = T // 512        # 8 token chunks for projections
    NK = L // 128        # 16 k tiles per sequence
    NQ = L // 512        # 4 q chunks per sequence

    with tile.TileContext(nc) as tc:
        with (
            tc.tile_pool(name="const", bufs=1) as constp,
            tc.tile_pool(name="weights", bufs=1) as wp,
            tc.tile_pool(name="qkv", bufs=1) as qkvp,
            tc.tile_pool(name="xin", bufs=3) as xp,
            tc.tile_pool(name="work", bufs=3) as workp,
            tc.tile_pool(name="vaug", bufs=20) as vaugp,
            tc.tile_pool(name="epi", bufs=4) as epip,
        ):
            tril_t = constp.tile([128, 128], F32, tag="tril")
            ident_t = constp.tile([128, 128], F32R, tag="ident")
            onescol_t = constp.tile([128, 1], F32R, tag="onescol")
            n30_t = constp.tile([128, 1], F32, tag="n30")
            nc.gpsimd.memset(n30_t[:], -TAU)
            identf_t = constp.tile([128, 128], F32, tag="identf")
            nc.sync.dma_start(identf_t[:].bitcast(F32R), ident_d.ap()[:])
            nc.sync.dma_start(tril_t[:], tril_d.ap()[:])
            nc.sync.dma_start(ident_t[:], ident_d.ap()[:])
            nc.sync.dma_start(onescol_t[:], onescol_d.ap()[:])

            # weight tiles: w[:, d*128:(d+1)*128] = W.T chunk d ([128, 128])
            w_tiles = []
            for name, d_in in (("wq", wq_d), ("wk", wk_d), ("wv", wv_d)):
                wt = wp.tile([128, ND * E], F32R, tag=name)
                nc.sync.dma_start(
                    wt[:].rearrange("p (d e) -> p d e", d=ND),
                    d_in.ap().rearrange("(d p) e -> p d e", p=128),
                )
                w_tiles.append(wt)

            # QKV storage, per head; rows 0:64 data, row 64 = augmentation
            QT = [qkvp.tile([65, T], F32R, tag=f"qt{h}", name=f"qt{h}")
                  for h in range(2)]
            KT = [qkvp.tile([65, T], F32R, tag=f"kt{h}", name=f"kt{h}")
                  for h in range(2)]
            VT = [qkvp.tile([64, T], F32R, tag=f"vt{h}", name=f"vt{h}")
                  for h in range(2)]
            for h in range(2):
                nc.sync.dma_start(QT[h][64:65, :], ones_d.ap()[:])
                nc.sync.dma_start(KT[h][64:65, :], kaug_d.ap()[:])

            # ---------------- Phase A: QKV projections ----------------
            with tc.tile_pool(name="psA", bufs=2, space="PSUM") as psA:
                for t in range(NT):
                    t0 = t * 512
                    ps = [psA.tile([128, 512], F32, tag=f"ps{p}", name=f"ps{p}")
                          for p in range(3)]
                    for d in range(ND):
                        xt = xp.tile([128, 512], F32R, tag="xt")
                        nc.sync.dma_start(
                            xt[:], xT_d.ap()[d * 128:(d + 1) * 128,
                                             t0:t0 + 512])
                        for p in range(3):
                            nc.tensor.matmul(
                                ps[p][:],
                                w_tiles[p][:, d * E:(d + 1) * E],
                                xt[:],
                                start=(d == 0), stop=(d == ND - 1),
                            )
                    for h in range(2):
                        sl = slice(h * 64, h * 64 + 64)
                        nc.vector.tensor_copy(QT[h][0:64, t0:t0 + 512],
                                              ps[0][sl, :])
                        nc.vector.tensor_copy(KT[h][0:64, t0:t0 + 512],
                                              ps[1][sl, :])
                        nc.vector.tensor_copy(VT[h][0:64, t0:t0 + 512],
                                              ps[2][sl, :])

            # ---------------- Phase B: attention per (b, h) ----------------
            with (
                tc.tile_pool(name="psS", bufs=4, space="PSUM") as psSp,
                tc.tile_pool(name="psO", bufs=2, space="PSUM") as psOp,
            ):
                for b in range(B):
                    for h in range(2):
                        tok0 = b * L
                        # V^T -> V tiles, augmented with ones column
                        vaug = []
                        for ki in range(NK):
                            pv = psSp.tile([128, 64], F32R, tag="psS")
                            nc.tensor.transpose(
                                pv[:],
                                VT[h][0:64, tok0 + ki * 128:tok0 + ki * 128 + 128],
                                ident_t[0:64, 0:64])
                            va = vaugp.tile([128, 65], F32R, tag="vaug")
                            nc.vector.tensor_copy(va[:, 0:64], pv[:])
                            nc.vector.tensor_copy(va[:, 64:65], onescol_t[:])
                            vaug.append(va)

                        for qc in range(NQ):
                            q0c = qc * 512
                            po = psOp.tile([65, 512], F32, tag="psO")
                            klast = 4 * qc + 3
                            for ki in range(klast + 1):
                                q0 = max(q0c, ki * 128)
                                qlen = q0c + 512 - q0
                                k0 = tok0 + ki * 128
                                pss = psSp.tile([128, 512], F32, tag="psS")
                                nc.tensor.matmul(
                                    pss[:, 0:qlen],
                                    KT[h][:, k0:k0 + 128],
                                    QT[h][:, tok0 + q0:tok0 + q0 + qlen],
                                    start=True, stop=True)
                                tt = workp.tile([128, 512], F32, tag="tanh")
                                nc.scalar.activation(
                                    tt[:, 0:qlen], pss[:, 0:qlen],
                                    AF.Tanh, scale=0.125)
                                pp = workp.tile([128, 512], F32R, tag="prob")
                                nc.scalar.activation(
                                    pp[:, 0:qlen], tt[:, 0:qlen],
                                    AF.Exp, bias=n30_t[:], scale=TAU)
                                if ki * 128 >= q0c:
                                    # diagonal block: keep q >= k
                                    nc.vector.tensor_mul(
                                        pp[:, 0:128], pp[:, 0:128], tril_t[:])
                                nc.tensor.matmul(
                                    po[:, q0 - q0c:512],
                                    vaug[ki][:],
                                    pp[:, 0:qlen],
                                    start=(ki == 0), stop=(ki == klast))

                            # epilogue: transpose back, normalize, store
                            ot = epip.tile([65, 512], F32, tag="ot")
                            nc.scalar.copy(ot[:], po[:])
                            for j in range(4):
                                pt = psSp.tile([128, 65], F32, tag="psS")
                                nc.tensor.transpose(
                                    pt[:], ot[:, j * 128:(j + 1) * 128],
                                    identf_t[0:65, 0:65])
                                of = epip.tile([128, 65], F32, tag="of")
                                nc.vector.tensor_copy(of[:], pt[:])
                                rec = epip.tile([128, 1], F32, tag="rec")
                                nc.vector.reciprocal(rec[:], of[:, 64:65])
                                o64 = epip.tile([128, 64], F32, tag="o64")
                                nc.vector.tensor_scalar_mul(
                                    o64[:], of[:, 0:64], rec[:])
                                q0 = q0c + j * 128
                                nc.sync.dma_start(
                                    out_d.ap()[b, q0:q0 + 128,
                                               h * 64:(h + 1) * 64],
                                    o64[:])

    nc.compile()
    return nc


def _get_program():
    if "nc" not in _CACHE:
        _CACHE["nc"] = _build_program()
    return _CACHE["nc"]


def _prep_inputs(input, attention_mask, W_Q, W_K, W_V):
    x = np.asarray(input, dtype=np.float32).reshape(T, D)
    xT = np.ascontiguousarray(x.T)                          # [D, T]
    mask = np.asarray(attention_mask).astype(np.float32).reshape(1, T)
    kaug = (mask - 1.0) * NEG_BIG                           # 0 keep, -1e6 drop
    onesrow = np.ones((1, T), dtype=np.float32)
    onescol = np.ones((128, 1), dtype=np.float32)
    tril = np.triu(np.ones((128, 128), dtype=np.float32))   # keep[k, q] = q >= k
    ident = np.eye(128, dtype=np.float32)

    common = {
        "xT": xT, "kaug": kaug, "onesrow": onesrow, "onescol": onescol,
        "tril": tril, "ident": ident,
    }
    in_maps = []
    for c in range(N_CORES):
        sl = slice(c * E, (c + 1) * E)
        in_maps.append({
            **common,
            "wq": np.ascontiguousarray(np.asarray(W_Q, np.float32)[sl, :].T),
            "wk": np.ascontiguousarray(np.asarray(W_K, np.float32)[sl, :].T),
            "wv": np.ascontiguousarray(np.asarray(W_V, np.float32)[sl, :].T),
        })
    return in_maps


def kernel(input, attention_mask, W_Q, W_K, W_V):
    from concourse.bass_utils import run_bass_kernel_spmd

    nc = _get_program()
    in_maps = _prep_inputs(input, attention_mask, W_Q, W_K, W_V)
    res = run_bass_kernel_spmd(nc, in_maps, list(range(N_CORES)))
    return np.concatenate([res.results[c]["out"] for c in range(N_CORES)],
                          axis=2)
